# revision 25
# baseline (speedup 1.0000x reference)
"""Multi-head attention (B=4, N=2048, E=1024, H=16, D=64) on 8 TRN2 NeuronCores.

Sharding: core c = (batch b = c//2, head-half hh = c%2). Each core computes,
for its batch, 8 heads worth of Q/K/V projections (a 512-column slice of
Wq/Wk/Wv), full-sequence attention for those heads, and the partial output
projection through the matching 512-row slice of Wo. The host sums the two
partial outputs per batch and adds the closed-form bias correction
(bv/512) @ Wo + bo (each softmax row sums to exactly 1/512 after the
reference's divide-by-E/2).

Layout: x is transposed once (TensorE); Q^T/K^T live [e_out, tok] so the
scores are computed transposed (S^T = K Q^T) with the softmax denominator
folded in as a ones-column of V; exp runs on ScalarE straight out of PSUM
(no max subtraction -- scores are ~N(0,8), fp32 exp never overflows).
Head pairs run concurrently on PE row halves 0-63/64-127 (tile_position),
sharing one [128, 1024] S^T PSUM tile so a single exp covers both heads.
Per-pair normalization is deferred into the next pair's loop so it stays
off the PE critical path.

Reference quirk handled here: scores are NOT scaled by 1/sqrt(d); the
softmax output is divided by E/2 = 512.
"""

import sys

if "/opt/trn_rl_repo" not in sys.path:
    sys.path.insert(0, "/opt/trn_rl_repo")

import numpy as np

B, N, E, H = 4, 2048, 1024, 16
D = E // H          # 64
P = 128             # partitions
EH = E // 2         # 512: per-core e_out slice
HL = 8              # heads per core
ECH = E // P        # 8 e_in chunks
OCH = EH // P       # 4 e_out chunks
KC = N // P         # 16 key/token tiles
QH = 4              # q quarters per head pass
QHW = N // QH       # 512
MV = 512            # moving free dim (PSUM bank limit: 512 fp32)

_CACHE = {}


def _build():
    import concourse.bass as bass
    import concourse.tile as tile
    from concourse import bacc, mybir
    from concourse.masks import make_identity

    f32 = mybir.dt.float32
    f16 = mybir.dt.float16
    bf16 = mybir.dt.bfloat16
    Exp = mybir.ActivationFunctionType.Exp
    mult = mybir.AluOpType.mult

    nc = bacc.Bacc("TRN2", target_bir_lowering=False, debug=False)

    x_d = nc.dram_tensor("x", [N, E], f32, kind="ExternalInput").ap()
    wq_d = nc.dram_tensor("wq", [E, EH], f32, kind="ExternalInput").ap()
    wk_d = nc.dram_tensor("wk", [E, EH], f32, kind="ExternalInput").ap()
    wv_d = nc.dram_tensor("wv", [E, EH], f32, kind="ExternalInput").ap()
    wo_d = nc.dram_tensor("wo", [EH, E], f32, kind="ExternalInput").ap()
    bq_d = nc.dram_tensor("bq", [EH], f32, kind="ExternalInput").ap()
    bk_d = nc.dram_tensor("bk", [EH], f32, kind="ExternalInput").ap()
    out_d = nc.dram_tensor("out", [N, E], f32, kind="ExternalOutput").ap()

    with tile.TileContext(nc) as tc:
        with (
            tc.tile_pool(name="persist", bufs=1) as persist,
            tc.tile_pool(name="wstage", bufs=2) as wstage,
            tc.tile_pool(name="xstage", bufs=3) as xstage,
            tc.tile_pool(name="pt_sb", bufs=6) as pt_sb,
            tc.tile_pool(name="small", bufs=2) as small,
            tc.tile_pool(name="ostage", bufs=3) as ostage,
        ):
            # ---- persistent SBUF tensors ----
            xT = persist.tile([P, ECH, N], f16, tag="xT")       # x^T
            qT = persist.tile([P, OCH, N], f16, tag="qT")       # (x Wq + bq)^T
            kT = persist.tile([P, OCH, N], f16, tag="kT")
            vaug = persist.tile([P, KC, HL, D + 1], bf16, tag="vaug")
            oT = persist.tile([P, OCH, N], f16, tag="oT")       # normalized O^T
            wq_s = persist.tile([P, ECH, EH], f16, tag="wq_s")
            wk_s = persist.tile([P, ECH, EH], f16, tag="wk_s")
            wv_s = persist.tile([P, ECH, EH], f16, tag="wv_s")
            wo_s = persist.tile([P, OCH, E], f16, tag="wo_s")
            bq_s = persist.tile([1, EH], f16, tag="bq_s")
            bk_s = persist.tile([1, EH], f16, tag="bk_s")
            ident = persist.tile([P, P], f16, tag="ident")
            ones64 = persist.tile([1, D], bf16, tag="ones64")
            ones_tok = persist.tile([1, MV], f16, tag="ones_tok")

            make_identity(nc, ident)
            nc.gpsimd.memset(ones64, 1.0)
            nc.gpsimd.memset(ones_tok, 1.0)
            # ones column of V_aug (feeds the softmax denominator row)
            nc.gpsimd.memset(vaug[:, :, :, D : D + 1], 1.0)

            # biases: natural single-partition load, cast to f16; applied
            # inside the projection matmuls as a rank-1 update
            bqs32 = wstage.tile([1, EH], f32, tag="bstage")
            nc.sync.dma_start(out=bqs32, in_=bq_d[None, :])
            nc.vector.tensor_copy(out=bq_s, in_=bqs32)
            bks32 = wstage.tile([1, EH], f32, tag="bstage")
            nc.sync.dma_start(out=bks32, in_=bk_d[None, :])
            nc.vector.tensor_copy(out=bk_s, in_=bks32)

            # ---- weights: DMA f32, cast to f16 ----
            for c in range(ECH):
                for w_dram, w_sb in ((wk_d, wk_s), (wv_d, wv_s), (wq_d, wq_s)):
                    ws = wstage.tile([P, EH], f32, tag="wstage")
                    nc.scalar.dma_start(out=ws, in_=w_dram[c * P : (c + 1) * P, :])
                    nc.vector.tensor_copy(out=w_sb[:, c, :], in_=ws)
            for c in range(OCH):
                ws = wstage.tile([P, E], f32, tag="wostage")
                nc.scalar.dma_start(out=ws, in_=wo_d[c * P : (c + 1) * P, :])
                nc.vector.tensor_copy(out=wo_s[:, c, :], in_=ws)

            with (
                tc.tile_pool(name="psA", bufs=2, space="PSUM") as psA,
                tc.tile_pool(name="psV", bufs=2, space="PSUM") as psV,
            ):
                # ---- phase A: x load, cast, TensorE transpose; V-proj
                # for tile t follows immediately (it only needs tile t) ----
                for t in range(KC):
                    xs = xstage.tile([P, E], f32, tag="xs")
                    nc.sync.dma_start(out=xs, in_=x_d[t * P : (t + 1) * P, :])
                    xc = xstage.tile([P, E], f16, tag="xc")
                    nc.vector.tensor_copy(out=xc, in_=xs)
                    for c in range(ECH):
                        pt = psA.tile([P, P], f16, tag="pt")
                        nc.tensor.transpose(pt, xc[:, c * P : (c + 1) * P], ident)
                        nc.scalar.copy(out=xT[:, c, t * P : (t + 1) * P], in_=pt)
                    pv = psV.tile([P, EH], f32, tag="pv")
                    for ci in range(ECH):
                        nc.tensor.matmul(
                            pv,
                            lhsT=xT[:, ci, t * P : (t + 1) * P],
                            rhs=wv_s[:, ci, :],
                            start=(ci == 0),
                            stop=(ci == ECH - 1),
                        )
                    nc.vector.tensor_copy(
                        out=vaug[:, t, :, 0:D],
                        in_=pv.rearrange("p (h d) -> p h d", h=HL),
                    )

            with (
                tc.tile_pool(name="psS", bufs=2, space="PSUM") as psS,
                tc.tile_pool(name="psO", bufs=4, space="PSUM") as psO,
            ):

                def proj_qk(w_sb, b_sb, dst, co, th):
                    sl = slice(th * MV, (th + 1) * MV)
                    ps = psO.tile([P, MV], f32, tag="po")
                    for ci in range(ECH):
                        nc.tensor.matmul(
                            ps,
                            lhsT=w_sb[:, ci, co * P : (co + 1) * P],
                            rhs=xT[:, ci, sl],
                            start=(ci == 0),
                            stop=False,
                        )
                    # rank-1 bias add: bias row (stationary) x ones row
                    nc.tensor.matmul(
                        ps,
                        lhsT=b_sb[:, co * P : (co + 1) * P],
                        rhs=ones_tok,
                        start=False,
                        stop=True,
                    )
                    nc.vector.tensor_copy(out=dst[:, co, sl], in_=ps)

                def drain_head(h, ocp, zrow, qq):
                    """Normalize one head's accumulated O^T from its SBUF
                    staging copy into oT. Emitted deep inside the NEXT
                    pair's loop so the PE's static order is not stalled."""
                    bp = (h % 2) * D
                    qsl = slice(qq * QHW, (qq + 1) * QHW)
                    rf = small.tile([1, QHW], f32, tag="rf")
                    nc.vector.reciprocal(rf, zrow)
                    rb = small.tile([1, QHW], bf16, tag="rb")
                    nc.vector.tensor_scalar_mul(rb, rf, 1.0 / (E / 2.0))
                    pob = psO.tile([P, QHW], f32, tag="po")
                    nc.tensor.matmul(
                        pob[0:D, :], lhsT=ones64, rhs=rb, start=True, stop=True
                    )
                    rbc = small.tile([D, QHW], bf16, tag="rbc")
                    nc.vector.tensor_copy(out=rbc, in_=pob[0:D, :])
                    nc.vector.tensor_tensor(
                        out=oT[bp : bp + D, h // 2, qsl],
                        in0=ocp,
                        in1=rbc,
                        op=mult,
                    )

                def s_pair_for(j, qq, kc):
                    qsl = slice(qq * QHW, (qq + 1) * QHW)
                    ss = psS.tile([P, 2 * QHW], f32, tag="ss")
                    ksl = slice(kc * P, (kc + 1) * P)
                    nc.tensor.matmul(
                        ss[:, 0:QHW],
                        lhsT=kT[0:D, j, ksl],
                        rhs=qT[0:D, j, qsl],
                        start=True,
                        stop=True,
                    )
                    nc.tensor.matmul(
                        ss[:, QHW : 2 * QHW],
                        lhsT=kT[D : 2 * D, j, ksl],
                        rhs=qT[D : 2 * D, j, qsl],
                        start=True,
                        stop=True,
                    )
                    return ss

                def attn_pair(j, qq, pending, preS, nxt):
                    """S^T/exp/O for heads (2j, 2j+1) on quarter qq. S-pairs
                    run two steps ahead of the O-pairs (and preload into the
                    NEXT pair at kc 14/15) so ScalarE's exp stream never
                    waits on the PE's static order; the previous pair's
                    normalization drains mid-loop."""
                    po_e = psO.tile([P, QHW], f32, tag="po")
                    po_o = psO.tile([P, QHW], f32, tag="po")
                    sss = (
                        preS
                        if preS is not None
                        else [s_pair_for(j, qq, 0), s_pair_for(j, qq, 1)]
                    )
                    nxtS = []
                    for kc in range(KC):
                        pT = pt_sb.tile([P, 2 * QHW], bf16, tag="pT")
                        nc.scalar.activation(pT, sss[kc], Exp)
                        if kc + 2 < KC:
                            sss.append(s_pair_for(j, qq, kc + 2))
                        nc.tensor.matmul(
                            po_e[0 : D + 1, :],
                            lhsT=vaug[:, kc, 2 * j, :],
                            rhs=pT[:, 0:QHW],
                            start=(kc == 0),
                            stop=(kc == KC - 1),
                        )
                        nc.tensor.matmul(
                            po_o[0 : D + 1, :],
                            lhsT=vaug[:, kc, 2 * j + 1, :],
                            rhs=pT[:, QHW : 2 * QHW],
                            start=(kc == 0),
                            stop=(kc == KC - 1),
                        )
                        if kc == 5 and pending:
                            for h, ocp, zrow, pqq in pending:
                                drain_head(h, ocp, zrow, pqq)
                            pending.clear()
                        if nxt is not None and kc >= KC - 2:
                            nq, njj = nxt
                            nxtS.append(s_pair_for(njj, nq, kc - (KC - 2)))
                    out = []
                    for h, po in ((2 * j, po_e), (2 * j + 1, po_o)):
                        # two quick copies free the PSUM accumulator;
                        # Z first so the reciprocal can start early
                        zrow = small.tile([1, QHW], f32, tag="zrow")
                        nc.vector.tensor_copy(out=zrow, in_=po[D : D + 1, :])
                        ocp = small.tile([D, QHW], bf16, tag="ocp")
                        nc.vector.tensor_copy(out=ocp, in_=po[0:D, :])
                        out.append((h, ocp, zrow, qq))
                    return out, nxtS

                def outproj(qq):
                    DW = 512
                    for t in range(qq * (KC // QH), (qq + 1) * (KC // QH)):
                        for eo in range(E // DW):
                            esl = slice(eo * DW, (eo + 1) * DW)
                            pod = psO.tile([P, DW], f32, tag="po")
                            for c in range(OCH):
                                nc.tensor.matmul(
                                    pod,
                                    lhsT=oT[:, c, t * P : (t + 1) * P],
                                    rhs=wo_s[:, c, esl],
                                    start=(c == 0),
                                    stop=(c == OCH - 1),
                                )
                            os_ = ostage.tile([P, DW], f32, tag="os")
                            nc.vector.tensor_copy(out=os_, in_=pod)
                            nc.sync.dma_start(
                                out=out_d[t * P : (t + 1) * P, esl], in_=os_
                            )

                # Flat pair schedule: projections are emitted one pair ahead
                # of the attention that consumes them, so the exp stream of
                # pair p is never behind projection work for pair p+1.
                emitted_K = set()
                emitted_Q = set()

                def ensure_proj(qq, j):
                    if j not in emitted_K:
                        for th in range(N // MV):
                            proj_qk(wk_s, bk_s, kT, j, th)
                        emitted_K.add(j)
                    if (qq, j) not in emitted_Q:
                        proj_qk(wq_s, bq_s, qT, j, qq)
                        emitted_Q.add((qq, j))

                pairs = [(qq, j) for qq in range(QH) for j in range(HL // 2)]
                ensure_proj(*pairs[0])
                pending, preS = [], None
                for idx, (qq, j) in enumerate(pairs):
                    nxt = pairs[idx + 1] if idx + 1 < len(pairs) else None
                    if nxt is not None:
                        ensure_proj(*nxt)
                    pending, preS = attn_pair(j, qq, pending, preS, nxt)
                    if j == 0 and qq >= 1:
                        outproj(qq - 1)
                for h, ocp, zrow, pqq in pending:
                    drain_head(h, ocp, zrow, pqq)
                pending.clear()
                outproj(QH - 1)
    nc.compile()
    return nc


def _get_nc():
    if "nc" not in _CACHE:
        _CACHE["nc"] = _build()
    return _CACHE["nc"]


def kernel(x, Wq, bq, Wk, bk, Wv, bv, Wo, bo):
    from concourse.bass_utils import run_bass_kernel_spmd

    x = np.asarray(x, dtype=np.float32)
    Wq = np.asarray(Wq, dtype=np.float32)
    Wk = np.asarray(Wk, dtype=np.float32)
    Wv = np.asarray(Wv, dtype=np.float32)
    Wo = np.asarray(Wo, dtype=np.float32)
    bq = np.asarray(bq, dtype=np.float32)
    bk = np.asarray(bk, dtype=np.float32)
    bv = np.asarray(bv, dtype=np.float32)
    bo = np.asarray(bo, dtype=np.float32)

    nc = _get_nc()
    in_maps = []
    for c in range(8):
        b, hh = divmod(c, 2)
        sl = slice(hh * EH, (hh + 1) * EH)
        in_maps.append(
            {
                "x": np.ascontiguousarray(x[b]),
                "wq": np.ascontiguousarray(Wq[:, sl]),
                "wk": np.ascontiguousarray(Wk[:, sl]),
                "wv": np.ascontiguousarray(Wv[:, sl]),
                "wo": np.ascontiguousarray(Wo[sl, :]),
                "bq": np.ascontiguousarray(bq[sl]),
                "bk": np.ascontiguousarray(bk[sl]),
            }
        )
    res = run_bass_kernel_spmd(nc, in_maps, list(range(8))).results

    # Exact bias correction: softmax rows sum to 1, so A rows sum to 1/512
    # and the V-bias term is the constant row (bv/512) @ Wo; bo likewise.
    corr = (
        bv.astype(np.float64) @ Wo.astype(np.float64) / (E / 2.0)
        + bo.astype(np.float64)
    ).astype(np.float32)

    out = np.empty((B, N, E), dtype=np.float32)
    for b in range(B):
        out[b] = res[2 * b]["out"] + res[2 * b + 1]["out"] + corr[None, :]
    return out
